# revision 20
# baseline (speedup 1.0000x reference)
"""DETR scene-graph predicate head on 8 Trainium2 NeuronCores.

Math: logits[l,b,r,:] = concat(hs[l,b,q_sub], hs[l,b,q_obj]) @ W_pred.T + b_pred
where q_sub/q_obj are derived from (tgt_perm inverse, relationships, src_indices)
— pure integer index math, done on host.

Kernel strategy (per (l,b) block, batch axis sharded 8 ways):
  phase A: psum = hs_block[101,256].T-chunks @ onehot[101,128]
           -> gathered pair representations, ALREADY TRANSPOSED (d on partitions),
           one matmul does gather+transpose at once (onehot col j = selector of
           query row q_sub[j] for j<64, q_obj[j-64] for j>=64).
  phase B: psum_out[64,51] += reprT_chunk.T @ W_chunk  (4 accumulating matmuls)
  bias add on DVE, store [64,51] contiguous.

hs_block and its onehot are packed into one [101, 384] DRAM row per block so
each block needs a single input DMA (keeps per-matmul sync-wait fan-in low).
"""

import sys

import numpy as np

L, B, Q1, D = 6, 256, 101, 256
M, R, P = 64, 64, 51
NCORES = 8
BLOC = B // NCORES          # images per core
NB = L * BLOC               # (layer, image) blocks per core
PK = D + 2 * R              # packed row width: 256 hs + 128 onehot
G = 16                      # blocks per DMA group
NG = NB // G                # groups per core

_CACHE = {}


def _build_program():
    import concourse.bacc as bacc
    import concourse.mybir as mybir
    import concourse.tile as tile
    from contextlib import ExitStack

    f32 = mybir.dt.float32
    bf16 = mybir.dt.bfloat16
    nc = bacc.Bacc("TRN2", target_bir_lowering=False, debug=False)

    # phase-B col-packing: blocks 2k / 2k+1 share the PE array via
    # tile_position (0,0)/(0,64); outputs land on psum partitions 0:64 /
    # 64:128 at column slot k -> group output is [128, (G//2)*P].
    GH = G // 2
    pk = nc.dram_tensor("pk", [NG, 128, G * PK], bf16, kind="ExternalInput").ap()
    wt = nc.dram_tensor("wt", [128, 4 * P], bf16, kind="ExternalInput").ap()
    bias = nc.dram_tensor("bias", [128, GH * P], f32, kind="ExternalInput").ap()
    out = nc.dram_tensor("out", [NG, 128, GH * P], f32, kind="ExternalOutput").ap()

    with tile.TileContext(nc) as tc, ExitStack() as ctx:
        const = ctx.enter_context(tc.tile_pool(name="const", bufs=1))
        inp = ctx.enter_context(tc.tile_pool(name="inp", bufs=5))
        rep = ctx.enter_context(tc.tile_pool(name="rep", bufs=6))
        outp = ctx.enter_context(tc.tile_pool(name="outp", bufs=3))
        psA = ctx.enter_context(tc.tile_pool(name="psA", bufs=4, space="PSUM"))
        psO = ctx.enter_context(tc.tile_pool(name="psO", bufs=2, space="PSUM"))

        wt_t = const.tile([128, 4 * P], bf16)
        nc.sync.dma_start(out=wt_t[:], in_=wt[:])
        bias_t = const.tile([128, GH * P], f32)
        nc.sync.dma_start(out=bias_t[:], in_=bias[:])

        # HAM warm-up: ~20 dense N=512 matmuls (~4.5us) push the PE clock
        # from 1.2 to 2.4 GHz before the steady LDW+MM flow begins; the
        # steady flow has no long PE-idle windows, so it stays warm.
        wu = const.tile([128, 512], bf16)
        nc.vector.memset(wu[:], 0.0)
        wps = psA.tile([128, 512], f32, tag="pAB")
        for _ in range(20):
            nc.tensor.matmul(out=wps[:], lhsT=wu[:, 0:128], rhs=wu[:],
                             start=True, stop=True)

        for g in range(NG):
            # one contiguous load per group of G blocks (bf16)
            pk_t = inp.tile([128, G * PK], bf16, tag="pk")
            nc.gpsimd.dma_start(out=pk_t[:], in_=pk[g])
            o_t = outp.tile([128, GH * P], f32, tag="o")
            # all G blocks' phase-B outputs share one psum bank tile
            pO = psO.tile([128, GH * P], f32, tag="pO")

            # pairs of blocks (2k, 2k+1) flow together: 4 gather matmuls into
            # one full psum bank, one cast copy, then 8 col-packed predicate
            # matmuls (left/right array halves run concurrently).
            for k in range(GH):
                j0, j1 = 2 * k, 2 * k + 1
                pAB = psA.tile([128, 512], f32, tag="pAB")
                for s, j in enumerate((j0, j1)):
                    hs_t = pk_t[0:Q1, j * PK:j * PK + D]
                    oh_t = pk_t[0:Q1, j * PK + D:(j + 1) * PK]
                    # pAB cols [s*256 : s*256+256]: [d-chunk0 | d-chunk1],
                    # each [sub 64 | obj 64]
                    nc.tensor.matmul(out=pAB[:, s * 256:s * 256 + 2 * R],
                                     lhsT=hs_t[:, 0:128], rhs=oh_t[:],
                                     start=True, stop=True)
                    nc.tensor.matmul(out=pAB[:, s * 256 + 2 * R:s * 256 + 4 * R],
                                     lhsT=hs_t[:, 128:256], rhs=oh_t[:],
                                     start=True, stop=True)
                bAB = rep.tile([128, 512], bf16, tag="bAB")
                if k % 2 == 0:
                    nc.vector.tensor_copy(out=bAB[:], in_=pAB[:])
                else:
                    nc.scalar.copy(out=bAB[:], in_=pAB[:])

                o0 = pO[0:R, k * P:(k + 1) * P]
                o1 = pO[R:2 * R, k * P:(k + 1) * P]
                for c, (lo, hi) in enumerate(
                        [(0, R), (2 * R, 3 * R), (R, 2 * R), (3 * R, 4 * R)]):
                    wch = wt_t[:, c * P:(c + 1) * P]
                    nc.tensor.matmul(out=o0, lhsT=bAB[:, lo:hi], rhs=wch,
                                     start=(c == 0), stop=(c == 3),
                                     tile_position=(0, 0))
                    nc.tensor.matmul(out=o1, lhsT=bAB[:, 256 + lo:256 + hi],
                                     rhs=wch,
                                     start=(c == 0), stop=(c == 3),
                                     tile_position=(0, 64))

            # one bias add for the whole group, one store per group
            nc.vector.tensor_add(out=o_t[:], in0=pO[:], in1=bias_t[:])
            nc.scalar.dma_start(out=out[g], in_=o_t[:])

    nc.compile()
    return nc


def _host_indices(src_indices, tgt_perm, relationships):
    """q_sub, q_obj: [L, B, R] int64 — matched query slot per relation."""
    src = np.asarray(src_indices, dtype=np.int64)
    tgt = np.asarray(tgt_perm, dtype=np.int64)
    rel = np.asarray(relationships, dtype=np.int64)

    # lookup[l, b, tgt[l, b, k]] = k
    lookup = np.empty((L, B, M), dtype=np.int64)
    li = np.arange(L)[:, None, None]
    bi = np.arange(B)[None, :, None]
    lookup[li, bi, tgt] = np.broadcast_to(np.arange(M), (L, B, M))

    sub_t = np.broadcast_to(rel[None, :, :, 0], (L, B, R))
    obj_t = np.broadcast_to(rel[None, :, :, 1], (L, B, R))
    pos_sub = np.take_along_axis(lookup, sub_t, axis=2)
    pos_obj = np.take_along_axis(lookup, obj_t, axis=2)
    q_sub = np.take_along_axis(src, pos_sub, axis=2)
    q_obj = np.take_along_axis(src, pos_obj, axis=2)
    return q_sub, q_obj


def _host_prepare(hs, src_indices, tgt_perm, relationships, W_pred, b_pred):
    """Build per-core input maps."""
    hs = np.asarray(hs, dtype=np.float32)
    W = np.asarray(W_pred, dtype=np.float32)
    b = np.asarray(b_pred, dtype=np.float32)

    q_sub, q_obj = _host_indices(src_indices, tgt_perm, relationships)
    q_cat = np.concatenate([q_sub, q_obj], axis=-1)          # [L, B, 2R]
    onehot = (np.arange(Q1)[None, None, :, None] == q_cat[:, :, None, :])
    onehot = onehot.astype(np.float32)                        # [L, B, Q1, 2R]

    import ml_dtypes
    bf16 = ml_dtypes.bfloat16

    packed = np.zeros((L, B, 128, PK), dtype=bf16)
    packed[:, :, :Q1, :D] = hs.astype(bf16)
    packed[:, :, :Q1, D:] = onehot

    # W_pred [P, 2D] -> Wt [2D, P] -> packed [128, 4*P] chunk-major
    wt_packed = np.ascontiguousarray(
        W.T.reshape(4, 128, P).transpose(1, 0, 2).reshape(128, 4 * P)
    ).astype(bf16)
    bias_b = np.ascontiguousarray(np.tile(b[None, :], (128, G // 2)))  # [128, GH*P]

    in_maps = []
    for c in range(NCORES):
        sl = slice(c * BLOC, (c + 1) * BLOC)
        pk_core = packed[:, sl].reshape(NB, 128, PK)
        # group-major layout: [NG, Q1, G*PK], block j of group at cols j*PK
        pk_core = np.ascontiguousarray(
            pk_core.reshape(NG, G, 128, PK).transpose(0, 2, 1, 3)
            .reshape(NG, 128, G * PK))
        in_maps.append({
            "pk": pk_core,
            "wt": wt_packed,
            "bias": bias_b,
        })
    return in_maps


def kernel(hs, src_indices, tgt_perm, relationships, W_pred, b_pred):
    if "concourse" not in sys.modules:
        try:
            import concourse  # noqa: F401
        except ImportError:
            sys.path.insert(0, "/opt/trn_rl_repo")
    from concourse import bass_utils

    in_maps = _host_prepare(hs, src_indices, tgt_perm, relationships,
                            W_pred, b_pred)
    if "nc" not in _CACHE:
        _CACHE["nc"] = _build_program()
    nc = _CACHE["nc"]

    res = bass_utils.run_bass_kernel_spmd(nc, in_maps, list(range(NCORES)))
    outs = []
    for c in range(NCORES):
        o = res.results[c]["out"]                      # [NG, 128, GH*P]
        o = o.reshape(NG, 2, R, G // 2, P).transpose(0, 3, 1, 2, 4)
        outs.append(o.reshape(L, BLOC, R, P))
    return np.concatenate(outs, axis=1)


# revision 23
# speedup vs baseline: 1.2383x; 1.2383x over previous
"""DETR scene-graph predicate head on 8 Trainium2 NeuronCores.

Math: logits[l,b,r,:] = concat(hs[l,b,q_sub], hs[l,b,q_obj]) @ W_pred.T + b_pred
where q_sub/q_obj are derived from (tgt_perm inverse, relationships,
src_indices) — pure integer index math, done on host.

Strategy (batch axis sharded 8 ways; L*B/8 = 192 (layer,image) blocks/core):
  - Host builds, per block, a [101, 384] bf16 row: hs_block [101, 256] next to
    a one-hot selector [101, 128] (col j selects query q_sub[j], j<64, or
    q_obj[j-64]). Blocks are packed in groups of G=8 into one padded
    [128, G*384] DMA (128 partitions keeps all 16 SDMA engines engaged;
    SWDGE/gpsimd queue — the HWDGE path runs at single-engine rate here).
  - Phase A (gather+transpose fused): pAB = hs_chunk.T @ onehot gives the
    gathered pair representation already d-on-partitions, two matmuls per
    block into one shared psum bank per block-pair, one DVE/ACT cast copy
    to bf16 (alternating engines).
  - Phase B: logits[r, p] accumulates 4 matmuls (2 d-chunks x sub/obj)
    against W_pred.T chunks; blocks 2k/2k+1 run concurrently in the left/
    right PE array halves via tile_position (0,0)/(0,64), outputs stacked on
    psum partitions 0:64/64:128 of one group-wide bank.
  - One bias add (DVE) + one store (scalar-queue DMA) per group; host
    unpacks the [NG, 128, 4*51] layout back to [L, B, R, P].
  - A ~4.5us dense-matmul preamble warms the PE clock (HAM) to 2.4 GHz.

hs and W_pred are bf16 on-chip (one-hot gather is exact in bf16; psum
accumulates f32), giving ~2.4e-3 relative error vs the f32 reference.
"""

import sys

import numpy as np

L, B, Q1, D = 6, 256, 101, 256
M, R, P = 64, 64, 51
NCORES = 8
BLOC = B // NCORES          # images per core
NB = L * BLOC               # (layer, image) blocks per core
PK = D + 2 * R              # packed row width: 256 hs + 128 onehot
G = 8                       # blocks per DMA group
NG = NB // G                # groups per core

_CACHE = {}


def _build_program():
    import concourse.bacc as bacc
    import concourse.mybir as mybir
    import concourse.tile as tile
    from contextlib import ExitStack

    f32 = mybir.dt.float32
    bf16 = mybir.dt.bfloat16
    nc = bacc.Bacc("TRN2", target_bir_lowering=False, debug=False)

    # phase-B col-packing: blocks 2k / 2k+1 share the PE array via
    # tile_position (0,0)/(0,64); outputs land on psum partitions 0:64 /
    # 64:128 at column slot k -> group output is [128, (G//2)*P].
    GH = G // 2
    pk = nc.dram_tensor("pk", [NG, 128, G * PK], bf16, kind="ExternalInput").ap()
    wt = nc.dram_tensor("wt", [128, 4 * P], bf16, kind="ExternalInput").ap()
    bias = nc.dram_tensor("bias", [128, GH * P], f32, kind="ExternalInput").ap()
    out = nc.dram_tensor("out", [NG, 128, GH * P], f32, kind="ExternalOutput").ap()

    with tile.TileContext(nc) as tc, ExitStack() as ctx:
        const = ctx.enter_context(tc.tile_pool(name="const", bufs=1))
        inp = ctx.enter_context(tc.tile_pool(name="inp", bufs=5))
        rep = ctx.enter_context(tc.tile_pool(name="rep", bufs=6))
        outp = ctx.enter_context(tc.tile_pool(name="outp", bufs=3))
        psA = ctx.enter_context(tc.tile_pool(name="psA", bufs=4, space="PSUM"))
        psO = ctx.enter_context(tc.tile_pool(name="psO", bufs=2, space="PSUM"))

        wt_t = const.tile([128, 4 * P], bf16)
        nc.sync.dma_start(out=wt_t[:], in_=wt[:])
        bias_t = const.tile([128, GH * P], f32)
        nc.sync.dma_start(out=bias_t[:], in_=bias[:])

        # HAM warm-up: dense N=512 matmuls push the PE clock 1.2 -> 2.4 GHz
        wu = const.tile([128, 512], bf16)
        nc.vector.memset(wu[:], 0.0)
        wps = psA.tile([128, 512], f32, tag="pAB")
        for _ in range(20):
            nc.tensor.matmul(out=wps[:], lhsT=wu[:, 0:128], rhs=wu[:],
                             start=True, stop=True)


        for g in range(NG):
            # one contiguous load per group of G blocks (bf16)
            pk_t = inp.tile([128, G * PK], bf16, tag="pk")
            nc.gpsimd.dma_start(out=pk_t[:], in_=pk[g])
            o_t = outp.tile([128, GH * P], f32, tag="o")
            # all G blocks' phase-B outputs share one psum bank tile
            pO = psO.tile([128, GH * P], f32, tag="pO")

            # pairs of blocks (2k, 2k+1) flow together: 4 gather matmuls into
            # one full psum bank, one cast copy, then 8 col-packed predicate
            # matmuls (left/right array halves run concurrently).
            for k in range(GH):
                j0, j1 = 2 * k, 2 * k + 1
                pAB = psA.tile([128, 512], f32, tag="pAB")
                for s, j in enumerate((j0, j1)):
                    hs_t = pk_t[0:Q1, j * PK:j * PK + D]
                    oh_t = pk_t[0:Q1, j * PK + D:(j + 1) * PK]
                    # pAB cols [s*256 : s*256+256]: [d-chunk0 | d-chunk1],
                    # each [sub 64 | obj 64]
                    nc.tensor.matmul(out=pAB[:, s * 256:s * 256 + 2 * R],
                                     lhsT=hs_t[:, 0:128], rhs=oh_t[:],
                                     start=True, stop=True)
                    nc.tensor.matmul(out=pAB[:, s * 256 + 2 * R:s * 256 + 4 * R],
                                     lhsT=hs_t[:, 128:256], rhs=oh_t[:],
                                     start=True, stop=True)
                bAB = rep.tile([128, 512], bf16, tag="bAB")
                if k % 2 == 0:
                    nc.vector.tensor_copy(out=bAB[:], in_=pAB[:])
                else:
                    nc.scalar.copy(out=bAB[:], in_=pAB[:])

                o0 = pO[0:R, k * P:(k + 1) * P]
                o1 = pO[R:2 * R, k * P:(k + 1) * P]
                for c, (lo, hi) in enumerate(
                        [(0, R), (2 * R, 3 * R), (R, 2 * R), (3 * R, 4 * R)]):
                    wch = wt_t[:, c * P:(c + 1) * P]
                    nc.tensor.matmul(out=o0, lhsT=bAB[:, lo:hi], rhs=wch,
                                     start=(c == 0), stop=(c == 3),
                                     tile_position=(0, 0))
                    nc.tensor.matmul(out=o1, lhsT=bAB[:, 256 + lo:256 + hi],
                                     rhs=wch,
                                     start=(c == 0), stop=(c == 3),
                                     tile_position=(0, 64))

            # one bias add for the whole group, one store per group
            nc.vector.tensor_add(out=o_t[:], in0=pO[:], in1=bias_t[:])
            nc.scalar.dma_start(out=out[g], in_=o_t[:])

    nc.compile()
    return nc


def _host_indices(src_indices, tgt_perm, relationships):
    """q_sub, q_obj: [L, B, R] int64 — matched query slot per relation."""
    src = np.asarray(src_indices, dtype=np.int64)
    tgt = np.asarray(tgt_perm, dtype=np.int64)
    rel = np.asarray(relationships, dtype=np.int64)

    # lookup[l, b, tgt[l, b, k]] = k
    lookup = np.empty((L, B, M), dtype=np.int64)
    li = np.arange(L)[:, None, None]
    bi = np.arange(B)[None, :, None]
    lookup[li, bi, tgt] = np.broadcast_to(np.arange(M), (L, B, M))

    sub_t = np.broadcast_to(rel[None, :, :, 0], (L, B, R))
    obj_t = np.broadcast_to(rel[None, :, :, 1], (L, B, R))
    pos_sub = np.take_along_axis(lookup, sub_t, axis=2)
    pos_obj = np.take_along_axis(lookup, obj_t, axis=2)
    q_sub = np.take_along_axis(src, pos_sub, axis=2)
    q_obj = np.take_along_axis(src, pos_obj, axis=2)
    return q_sub, q_obj


def _host_prepare(hs, src_indices, tgt_perm, relationships, W_pred, b_pred):
    """Build per-core input maps."""
    hs = np.asarray(hs, dtype=np.float32)
    W = np.asarray(W_pred, dtype=np.float32)
    b = np.asarray(b_pred, dtype=np.float32)

    q_sub, q_obj = _host_indices(src_indices, tgt_perm, relationships)
    q_cat = np.concatenate([q_sub, q_obj], axis=-1)          # [L, B, 2R]
    onehot = (np.arange(Q1)[None, None, :, None] == q_cat[:, :, None, :])
    onehot = onehot.astype(np.float32)                        # [L, B, Q1, 2R]

    import ml_dtypes
    bf16 = ml_dtypes.bfloat16

    packed = np.zeros((L, B, 128, PK), dtype=bf16)
    packed[:, :, :Q1, :D] = hs.astype(bf16)
    packed[:, :, :Q1, D:] = onehot

    # W_pred [P, 2D] -> Wt [2D, P] -> packed [128, 4*P] chunk-major
    wt_packed = np.ascontiguousarray(
        W.T.reshape(4, 128, P).transpose(1, 0, 2).reshape(128, 4 * P)
    ).astype(bf16)
    bias_b = np.ascontiguousarray(np.tile(b[None, :], (128, G // 2)))  # [128, GH*P]

    in_maps = []
    for c in range(NCORES):
        sl = slice(c * BLOC, (c + 1) * BLOC)
        pk_core = packed[:, sl].reshape(NB, 128, PK)
        # group-major layout: [NG, Q1, G*PK], block j of group at cols j*PK
        pk_core = np.ascontiguousarray(
            pk_core.reshape(NG, G, 128, PK).transpose(0, 2, 1, 3)
            .reshape(NG, 128, G * PK))
        in_maps.append({
            "pk": pk_core,
            "wt": wt_packed,
            "bias": bias_b,
        })
    return in_maps


def kernel(hs, src_indices, tgt_perm, relationships, W_pred, b_pred):
    if "concourse" not in sys.modules:
        try:
            import concourse  # noqa: F401
        except ImportError:
            sys.path.insert(0, "/opt/trn_rl_repo")
    from concourse import bass_utils

    in_maps = _host_prepare(hs, src_indices, tgt_perm, relationships,
                            W_pred, b_pred)
    if "nc" not in _CACHE:
        _CACHE["nc"] = _build_program()
    nc = _CACHE["nc"]

    res = bass_utils.run_bass_kernel_spmd(nc, in_maps, list(range(NCORES)))
    outs = []
    for c in range(NCORES):
        o = res.results[c]["out"]                      # [NG, 128, GH*P]
        o = o.reshape(NG, 2, R, G // 2, P).transpose(0, 3, 1, 2, 4)
        outs.append(o.reshape(L, BLOC, R, P))
    return np.concatenate(outs, axis=1)
